# revision 30
# baseline (speedup 1.0000x reference)
"""Weighted-Dice-loss (nn_DiceLoss) Trainium2 Bass kernel.

Full inputs: pred [64,1,512,512] f32, mask [64,1,512,512] f32.
Output: scalar f32 = mean over images of 1 - (2*inter+0.5)/(union+0.5) with
  weit  = 1 + 5*|boxavg31(mask) - mask|
  inter = sum(sigmoid(pred)*mask*weit),  union = sum((sigmoid(pred)+mask)*weit)

Sharding: pure data parallel, 8 images per NeuronCore; per-image partial sums
come back per-core and the final (tiny) reduction happens on the host.

Per-core pipeline, per image (layout [128 partitions = H rows, free = W];
one-image software-pipeline skew keeps TensorE fed):
  DMA   mask -> [128, 4, 544] zero-padded tile (f32), pred -> [128, 4, 512]
  ACT   m9 = bf16(961*mask)   (for the identity matmuls below)
  DVE   W-axis 31-box in ONE sliding-window scan over the padded mask:
          state_t = (Mp[t+31] + state) - Mp[t], initial = sum(Mp[0:31])
        so u[j, w] = state at t = j*544 + w (bf16); the >=31 zero-pad
        columns between row-blocks keep windows from mixing rows and give
        count_include_pad edge clamping for free
  PE    4 PSUM banks accumulate  d961 = boxsum2d - 961*mask  via the 31-band
        H-axis matmul over u (three bf16 0/1 stationaries: main band +
        two corner blocks) plus (-I) @ m9
  ACT   a = Abs((5/961)*d961) -> bf16  (so weit = 1 + a)
        p = Sigmoid(pred) -> bf16, accum_out -> sum(p)
  DVE   s2 = (a+1)*mask  (scalar_tensor_tensor, free accum -> sum(mask*weit))
        pa = p*a   (TT bf16 2x;  sum(p*weit) = sum(p) + sum(pa))
        t  = s2*p  (TT bf16 2x;  = p*mask*weit)
  PE    ones-matmul reductions of pa and t into a [1, 1024] PSUM tile
  ACT   [1, 1024] PSUM -> SBUF row (DMA cannot read PSUM)
  DMA   accumulator tile [128, 16] and the [1, 8192] reduction row -> DRAM
Host: per-image inter/union from the partials, wdiss, mean over 64 images.

Measured on 8 axon NeuronCores: ~101 us HW exec, rel err ~1.8e-5
(per-core HBM roofline for the 16.8 MB of input is ~47 us; VectorE -- the
scan + three product passes -- is the binding engine at ~81 us busy).
"""

import numpy as np
import ml_dtypes
from contextlib import ExitStack

import concourse.tile as tile
from concourse import bacc, mybir
from concourse.bass_utils import run_bass_kernel_spmd

N_CORES = 8
B_PER_CORE = 8
H = W = 512
PB = 128          # SBUF partitions
NJ = H // PB      # 4 row-blocks per image
PADW = 16 + W + 16  # 544: padded row for the scan
KHALF = 15        # box radius
KK = 961.0        # 31*31

f32 = mybir.dt.float32
bf16 = mybir.dt.bfloat16
Alu = mybir.AluOpType
Act = mybir.ActivationFunctionType


def _host_constants():
    """Band stationaries [128, 3, 128] bf16 (exact 0/1) + identity f32.

    out_chunk[c, w] += sum_r lhsT[r, c] * rhs_chunk[r, w] with
      slot 0: C_below (k-chunk = m-chunk-1):  1 iff r - c >= 113
      slot 1: C_mid   (k-chunk = m-chunk):    1 iff |r - c| <= 15
      slot 2: C_above (k-chunk = m-chunk+1):  1 iff c - r >= 113
    """
    r = np.arange(PB)[:, None]
    c = np.arange(PB)[None, :]
    cb = np.zeros((PB, 3, PB), dtype=np.float32)
    cb[:, 0, :] = (r - c >= PB - KHALF)
    cb[:, 1, :] = (np.abs(r - c) <= KHALF)
    cb[:, 2, :] = (c - r >= PB - KHALF)
    negi = (-np.eye(PB)).astype(ml_dtypes.bfloat16)
    return cb.astype(ml_dtypes.bfloat16), negi


def _build():
    nc = bacc.Bacc("TRN2", target_bir_lowering=False, debug=False,
                   num_devices=N_CORES)
    pred_d = nc.dram_tensor("pred", [B_PER_CORE, H, W], f32, kind="ExternalInput")
    mask_d = nc.dram_tensor("mask", [B_PER_CORE, H, W], f32, kind="ExternalInput")
    band_d = nc.dram_tensor("band", [PB, 3, PB], bf16, kind="ExternalInput")
    negi_d = nc.dram_tensor("negi", [PB, PB], bf16, kind="ExternalInput")
    acc_d = nc.dram_tensor("acc", [PB, B_PER_CORE * 2], f32, kind="ExternalOutput")
    red_d = nc.dram_tensor("red", [1, B_PER_CORE * 2 * W], f32, kind="ExternalOutput")

    pred_r = pred_d.ap().rearrange("b (j p) w -> b p j w", p=PB)
    mask_r = mask_d.ap().rearrange("b (j p) w -> b p j w", p=PB)

    with tile.TileContext(nc) as tc:
        with ExitStack() as ctx:
            cpool = ctx.enter_context(tc.tile_pool(name="cpool", bufs=1))
            ppool = ctx.enter_context(tc.tile_pool(name="ppool", bufs=3))
            m9pool = ctx.enter_context(tc.tile_pool(name="m9pool", bufs=2))
            upool = ctx.enter_context(tc.tile_pool(name="upool", bufs=3))
            apool = ctx.enter_context(tc.tile_pool(name="apool", bufs=3))
            sigpool = ctx.enter_context(tc.tile_pool(name="sigpool", bufs=3))
            scr = ctx.enter_context(tc.tile_pool(name="scr", bufs=2))
            pspool = ctx.enter_context(tc.tile_pool(name="pspool", bufs=1, space="PSUM"))
            redpool = ctx.enter_context(tc.tile_pool(name="redpool", bufs=2, space="PSUM"))

            cb = cpool.tile([PB, 3, PB], bf16, name="cb")
            nc.sync.dma_start(cb[:], band_d.ap())
            negi = cpool.tile([PB, PB], bf16, name="negi")
            nc.sync.dma_start(negi[:], negi_d.ap())
            zcol = cpool.tile([PB, 1], f32, name="zcol")
            nc.vector.memset(zcol[:], 0.0)
            ones = cpool.tile([PB, 1], bf16, name="ones")
            nc.vector.memset(ones[:], 1.0)

            acc = cpool.tile([PB, B_PER_CORE * 2], f32, name="acc")
            redsb = cpool.tile([1, B_PER_CORE * 2 * W], f32, name="redsb")

            # persistent mask tiles (4-deep rotation); pad columns zeroed
            # once here and never written again -- DMAs only touch the valid
            # columns, so the zero pads survive reuse.
            mp_tiles = []
            for i in range(4):
                mpt = cpool.tile([PB, NJ * PADW], f32, name=f"mpt{i}")
                m3 = mpt.rearrange("p (j w) -> p j w", j=NJ)
                nc.gpsimd.memset(m3[:, :, 0:16], 0.0)
                nc.gpsimd.memset(m3[:, :, 16 + W:PADW], 0.0)
                mp_tiles.append(mpt)

            # one-image software pipeline: stage A(b) produces u/a/sg for
            # image b (keeps PE fed with band matmuls early); stage B(b-1)
            # runs the DVE products + PE ones-reductions for the previous
            # image while stage A(b+1)'s scan/diff proceed.
            stash = {}
            for b in range(B_PER_CORE + 1):
                if b < B_PER_CORE:
                    # ---- stage A: load, W-box, H-box matmuls, ACT ----
                    mp = mp_tiles[b % 4]
                    mp3 = mp.rearrange("p (j w) -> p j w", j=NJ)
                    nc.sync.dma_start(mp3[:, :, 16:16 + W], mask_r[b])

                    # pred on the SWDGE (gpsimd) queue so mask DMAs don't
                    # queue behind it on the sync HWDGE ring
                    pt = ppool.tile([PB, NJ * W], f32, name="pt")
                    pt3 = pt.rearrange("p (j w) -> p j w", j=NJ)
                    nc.gpsimd.dma_start(pt3[:], pred_r[b])

                    # 961*mask in bf16 for the ident matmuls (ACT has slack)
                    m9 = m9pool.tile([PB, NJ * W], bf16, name="m9")
                    m93 = m9.rearrange("p (j w) -> p j w", j=NJ)
                    nc.scalar.activation(m93[:], mp3[:, :, 16:16 + W],
                                         Act.Copy, bias=0.0, scale=KK)

                    # W-axis 31-box as ONE sliding-window scan:
                    #   state_t = (Mp[t+31] + state) - Mp[t]
                    # With initial = sum(Mp[0:31]), state_t = sum Mp(t+1..t+31],
                    # i.e. u[j, w] = state at t = j*544 + w.  The >=31 zero-pad
                    # columns between row-blocks keep windows from mixing rows.
                    SCANL = (NJ - 1) * PADW + W  # 2144
                    init = upool.tile([PB, 1], f32, name="init", tag="init")
                    nc.vector.tensor_reduce(init[:], mp[:, 0:31],
                                            mybir.AxisListType.X, Alu.add)
                    u = upool.tile([PB, SCANL], bf16, name="u")
                    nc.vector.tensor_tensor_scan(
                        u[:], mp[:, 31:31 + SCANL], mp[:, 0:SCANL],
                        init[:], Alu.add, Alu.subtract)

                    ps = pspool.tile([PB, NJ * W], f32, name="ps")
                    ps3 = ps.rearrange("p (j w) -> p j w", j=NJ)
                    groups = [
                        (1, [(j, j) for j in range(NJ)]),                # C_mid
                        (0, [(j, j - 1) for j in range(1, NJ)]),         # C_below
                        (2, [(j, j + 1) for j in range(NJ - 1)]),        # C_above
                    ]
                    for slot, pairs in groups:
                        for j, i in pairs:
                            nc.tensor.matmul(
                                ps3[:, j, :], cb[:, slot, :],
                                u[:, i * PADW:i * PADW + W],
                                start=(slot == 1), stop=False)
                    for j in range(NJ):
                        nc.tensor.matmul(
                            ps3[:, j, :], negi[:], m93[:, j, :],
                            start=False, stop=True)

                    col = 2 * b
                    a = apool.tile([PB, NJ * W], bf16, name="a")
                    nc.scalar.activation(a[:], ps[:], Act.Abs,
                                         bias=0.0, scale=5.0 / KK)
                    sg = sigpool.tile([PB, NJ * W], bf16, name="sg")
                    nc.scalar.activation(sg[:], pt[:], Act.Sigmoid,
                                         accum_out=acc[:, col + 1:col + 2])
                    stash[b] = (mp3, a, sg)

                if b >= 1:
                    # ---- stage B: products + reductions for image b-1 ----
                    bb = b - 1
                    mp3p, ap, sgp = stash.pop(bb)
                    col = 2 * bb
                    s2 = scr.tile([PB, NJ * W], bf16, name="s2", tag="s2")
                    nc.vector.scalar_tensor_tensor(
                        s2[:], ap[:], 1.0, mp3p[:, :, 16:16 + W],
                        Alu.add, Alu.mult, accum_out=acc[:, col:col + 1])
                    pa = scr.tile([PB, NJ * W], bf16, name="pa", tag="pa")
                    nc.vector.tensor_tensor(pa[:], sgp[:], ap[:], Alu.mult)
                    tt = scr.tile([PB, NJ * W], bf16, name="tt", tag="tt")
                    nc.vector.tensor_tensor(tt[:], s2[:], sgp[:], Alu.mult)

                    red = redpool.tile([1, 2 * W], f32, name="red")
                    pa3 = pa.rearrange("p (j w) -> p j w", j=NJ)
                    tt3 = tt.rearrange("p (j w) -> p j w", j=NJ)
                    for j in range(NJ):
                        nc.tensor.matmul(red[:, 0:W], ones[:], pa3[:, j, :],
                                         start=(j == 0), stop=(j == NJ - 1))
                    for j in range(NJ):
                        nc.tensor.matmul(red[:, W:2 * W], ones[:], tt3[:, j, :],
                                         start=(j == 0), stop=(j == NJ - 1))
                    nc.scalar.copy(redsb[:, bb * 2 * W:(bb + 1) * 2 * W], red[:])

            nc.sync.dma_start(acc_d.ap(), acc[:])
            nc.sync.dma_start(red_d.ap(), redsb[:])

    nc.compile()
    return nc


_NC = None


def _get_nc():
    global _NC
    if _NC is None:
        _NC = _build()
    return _NC


def _in_maps(pred, mask):
    band, negi = _host_constants()
    ims = []
    for c in range(N_CORES):
        sl = slice(c * B_PER_CORE, (c + 1) * B_PER_CORE)
        ims.append({
            "pred": np.ascontiguousarray(
                pred[sl].reshape(B_PER_CORE, H, W).astype(np.float32)),
            "mask": np.ascontiguousarray(
                mask[sl].reshape(B_PER_CORE, H, W).astype(np.float32)),
            "band": band,
            "negi": negi,
        })
    return ims


def _host_reduce(results):
    """per-core acc [128, 16] + red [8, 1024] f32 -> final scalar loss."""
    wd = []
    for r in results:
        a = r["acc"].astype(np.float64)
        rd = r["red"].astype(np.float64).reshape(B_PER_CORE, 2 * W)
        for b in range(B_PER_CORE):
            s2 = a[:, 2 * b + 0].sum()       # sum(mask*weit)
            psum = a[:, 2 * b + 1].sum()     # sum(p)
            pasum = rd[b, 0:W].sum()         # sum(p*(weit-1))
            t = rd[b, W:2 * W].sum()         # sum(p*mask*weit)
            inter = t
            union = (psum + pasum) + s2
            wd.append(1.0 - (2.0 * inter + 0.5) / (union + 0.5))
    return np.array(np.mean(wd), dtype=np.float32)


def kernel(pred, mask):
    nc = _get_nc()
    res = run_bass_kernel_spmd(nc, _in_maps(pred, mask),
                               core_ids=list(range(N_CORES)))
    return _host_reduce(res.results)
